# revision 1
# baseline (speedup 1.0000x reference)
"""Bidirectional Mamba on 8 Trainium2 NeuronCores.

Sharding: 8 cores = (2 directions) x (4 batch elements); each core runs one
full Mamba block on its (L=1024, DM=512) sequence. The backward direction is
handled by flipping the sequence on the host before/after, so all cores run
the identical SPMD program with different data.

Per-core layout: channels d on partitions, time t on the free dim. The d=512
channels form 4 chunks of 128; chunk pairs are concatenated along the free
dim into (128, 2048) tiles so each vector-engine op amortizes its fixed
overhead over 2048 elements. The selective scan is one tensor_tensor_scan
per (chunk-pair, n) with a zeroed decay column at the pair boundary (state
fp32). B rows broadcast across partitions via K=1 PE matmuls into PSUM;
C rows broadcast via partition-replicating DMA from a DRAM copy of dbc;
y = sum_n h_n*C_n accumulates in PSUM via identity matmuls (float32r) so the
vector engine only carries the scan and the dBx multiply, and the h*C
multiply runs on GPSIMD.
"""
import contextlib

import numpy as np

import concourse.bacc as bacc
import concourse.tile as tile
import concourse.mybir as mybir
from concourse.bass_utils import run_bass_kernel_spmd

F32 = mybir.dt.float32
F32R = mybir.dt.float32r
AF = mybir.ActivationFunctionType
OP = mybir.AluOpType

DM = 512
DI = 512
L = 1024
N = 16
K = 4
R = 32
P = 128
NCH = DI // P          # 4 d-chunks
W = 2 * L              # wide tile free size (chunk pair)
TB = 512               # t-block for matmul moving operand
NTB = L // TB          # 2
N_CORES = 8

USE_F32R_GEMM = False  # plain fp32 GEMMs (exact; PE has headroom)
USE_F32R_AUX = True    # broadcasts + y-accumulate in float32r


def _mm(nc, out, lhsT, rhs, start, stop, f32r):
    if f32r:
        lhsT = lhsT.bitcast(F32R)
        rhs = rhs.bitcast(F32R)
    nc.tensor.matmul(out, lhsT=lhsT, rhs=rhs, start=start, stop=stop,
                     skip_group_check=True)


def emit_mamba(tc, io):
    nc = tc.nc
    f32 = F32

    with contextlib.ExitStack() as ctx:
        # ---- persistent SBUF tiles ----
        per = ctx.enter_context(tc.tile_pool(name="per", bufs=1))

        def ptile(tag, shape, dtype=f32):
            return per.tile(shape, dtype, tag=tag, name=tag)

        Wc_sb = [ptile(f"Wc{i}", [P, K]) for i in range(NCH)]
        bconv_sb = [ptile(f"bcv{i}", [P, 1]) for i in range(NCH)]
        Wx_sb = [ptile(f"Wx{i}", [P, R + 2 * N]) for i in range(NCH)]
        Wdt_sb = ptile("Wdt", [R, DI])
        bdt_sb = [ptile(f"bdt{i}", [P, 1]) for i in range(NCH)]
        A_sb = [ptile(f"A{i}", [P, N]) for i in range(NCH)]
        D_sb = [ptile(f"D{i}", [P, 1]) for i in range(NCH)]
        Wout_sb = [ptile(f"Wo{i}", [P, DM]) for i in range(NCH)]
        ones_sb = ptile("ones", [1, P], F32R)
        ident_sb = ptile("ident", [P, P], F32R)
        zcol_sb = ptile("zcol", [P, 1])
        nc.vector.memset(zcol_sb[:], 0.0)

        # chunk pair h covers chunks (2h, 2h+1); chunk dc sits at columns
        # (dc%2)*L : (dc%2+1)*L of wide tile h = dc//2
        xc_sb = [ptile(f"xc{i}", [P, W]) for i in range(2)]
        zs_sb = [ptile(f"zs{i}", [P, W]) for i in range(2)]
        xs_sb = [ptile(f"xs{i}", [P, W]) for i in range(2)]
        dt_sb = [ptile(f"dt{i}", [P, W]) for i in range(2)]
        u_sb = [ptile(f"u{i}", [P, W]) for i in range(2)]
        yz_sb = [ptile(f"yz{i}", [P, W]) for i in range(2)]
        dbc_sb = ptile("dbc", [R + 2 * N, L])

        def wide(arr, dc, lo=0, hi=L):
            return arr[dc // 2][:, (dc % 2) * L + lo : (dc % 2) * L + hi]

        for i in range(NCH):
            sl = slice(i * P, (i + 1) * P)
            nc.sync.dma_start(Wc_sb[i][:], io["Wc"][sl, :])
            nc.sync.dma_start(bconv_sb[i][:], io["bconv"][sl, :])
            nc.sync.dma_start(Wx_sb[i][:], io["Wx"][sl, :])
            nc.sync.dma_start(bdt_sb[i][:], io["bdt"][sl, :])
            nc.sync.dma_start(A_sb[i][:], io["A_sc"][sl, :])
            nc.sync.dma_start(D_sb[i][:], io["Dv"][sl, :])
            nc.sync.dma_start(Wout_sb[i][:], io["W_out"][sl, :])
        nc.sync.dma_start(Wdt_sb[:], io["Wdt"][:, :])
        nc.sync.dma_start(ones_sb[:], io["ones"][:, :])
        nc.sync.dma_start(ident_sb[:], io["ident"][:, :])

        # ---- GEMM A: xz_T = W_in^T @ x_T ; silu on z half ----
        with tc.tile_pool(name="gin", bufs=1) as gin, tc.tile_pool(
            name="psA", bufs=4, space="PSUM"
        ) as psA:
            W_in_sb = [
                gin.tile([P, 2 * DI], f32, tag=f"Wi{i}", name=f"Wi{i}")
                for i in range(NCH)
            ]
            xT_sb = [
                gin.tile([P, L], f32, tag=f"xT{i}", name=f"xT{i}")
                for i in range(NCH)
            ]
            for i in range(NCH):
                sl = slice(i * P, (i + 1) * P)
                nc.sync.dma_start(W_in_sb[i][:], io["W_in"][sl, :])
                nc.sync.dma_start(xT_sb[i][:], io["xT"][sl, :])

            for cb in range(2 * DI // P):  # 8 output blocks of 128 channels
                for tb in range(NTB):
                    ps = psA.tile([P, TB], f32, tag="psA", name="psA")
                    for mk in range(NCH):
                        _mm(
                            nc, ps[:],
                            W_in_sb[mk][:, cb * P : (cb + 1) * P],
                            xT_sb[mk][:, tb * TB : (tb + 1) * TB],
                            start=(mk == 0), stop=(mk == NCH - 1),
                            f32r=USE_F32R_GEMM,
                        )
                    lo, hi = tb * TB, (tb + 1) * TB
                    if cb < NCH:
                        nc.scalar.activation(wide(xc_sb, cb, lo, hi), ps[:], AF.Copy)
                    else:
                        nc.scalar.activation(
                            wide(zs_sb, cb - NCH, lo, hi), ps[:], AF.Silu
                        )

        # ---- causal depthwise conv (K=4) + silu -> xs ----
        with tc.tile_pool(name="cv", bufs=2) as cvp:
            for dc in range(NCH):
                xcv = cvp.tile([P, L], f32, tag="xcv", name="xcv")
                nc.vector.tensor_scalar_mul(xcv[:], wide(xc_sb, dc), Wc_sb[dc][:, 3:4])
                for k in (2, 1, 0):
                    s = K - 1 - k
                    nc.vector.scalar_tensor_tensor(
                        out=xcv[:, s:],
                        in0=wide(xc_sb, dc, 0, L - s),
                        scalar=Wc_sb[dc][:, k : k + 1],
                        in1=xcv[:, s:],
                        op0=OP.mult,
                        op1=OP.add,
                    )
                nc.scalar.activation(
                    wide(xs_sb, dc), xcv[:], AF.Silu, bias=bconv_sb[dc][:, 0:1]
                )

        # ---- GEMM B: dbc_T = W_xproj^T @ xs_T  (64 rows: dt_in | B | C) ----
        with tc.tile_pool(name="psB", bufs=2, space="PSUM") as psB:
            for tb in range(NTB):
                ps = psB.tile([R + 2 * N, TB], f32, tag="psB", name="psB")
                for dc in range(NCH):
                    _mm(
                        nc, ps[:], Wx_sb[dc][:],
                        wide(xs_sb, dc, tb * TB, (tb + 1) * TB),
                        start=(dc == 0), stop=(dc == NCH - 1),
                        f32r=USE_F32R_GEMM,
                    )
                nc.scalar.activation(
                    dbc_sb[:, tb * TB : (tb + 1) * TB], ps[:], AF.Copy
                )

        # ---- GEMM C: dt_T = softplus(W_dt^T @ dt_in_T + b_dt) ----
        # softplus(x) = ln(1 + exp(x)); the ACT softplus table isn't available,
        # but exp and ln share one table set (natural_log_exp_and_others).
        with tc.tile_pool(name="psC", bufs=2, space="PSUM") as psC, tc.tile_pool(
            name="spl", bufs=2
        ) as spl:
            for dc in range(NCH):
                for tb in range(NTB):
                    ps = psC.tile([P, TB], f32, tag="psC", name="psC")
                    _mm(
                        nc, ps[:], Wdt_sb[:, dc * P : (dc + 1) * P],
                        dbc_sb[0:R, tb * TB : (tb + 1) * TB],
                        start=True, stop=True, f32r=USE_F32R_GEMM,
                    )
                    et = spl.tile([P, TB], f32, tag="et", name="et")
                    nc.scalar.activation(
                        et[:], ps[:], AF.Exp, bias=bdt_sb[dc][:, 0:1]
                    )
                    nc.scalar.activation(
                        wide(dt_sb, dc, tb * TB, (tb + 1) * TB),
                        et[:],
                        AF.Ln,
                        bias=1.0,
                    )

        # u = dt * xs (wide)
        for h in range(2):
            nc.vector.tensor_mul(u_sb[h][:], dt_sb[h][:], xs_sb[h][:])

        # dbc copy in DRAM for DMA partition-broadcast of C rows
        nc.sync.dma_start(io["dbc_dram"][:, :], dbc_sb[:])

        # ---- selective scan: per chunk pair, n inner ----
        with tc.tile_pool(name="scan", bufs=3) as sp, tc.tile_pool(
            name="rowp", bufs=2
        ) as rowp, tc.tile_pool(name="cbp", bufs=2) as cbp, tc.tile_pool(
            name="psbc", bufs=2, space="PSUM"
        ) as psbc, tc.tile_pool(name="psy", bufs=1, space="PSUM") as psy:

            def build_bb(n):
                # stage the B row at partition 0, then PE-broadcast to PSUM
                Brow = rowp.tile([1, L], F32R, tag="Brow", name="Brow")
                nc.sync.dma_start(
                    Brow[:], dbc_sb[R + n : R + n + 1, :].bitcast(F32R)
                )
                Bb = psbc.tile([P, L], f32, tag="Bb", name="Bb")
                for tb in range(NTB):
                    tsl = slice(tb * TB, (tb + 1) * TB)
                    _mm(nc, Bb[:, tsl], ones_sb[:], Brow[:, tsl],
                        start=True, stop=True, f32r=USE_F32R_AUX)
                return Bb

            def build_cb(n):
                Cb = cbp.tile([P, L], f32, tag="Cb", name="Cb")
                nc.sync.dma_start(
                    Cb[:],
                    io["dbc_dram"][R + N + n : R + N + n + 1, :]
                    .partition_broadcast(P),
                )
                return Cb

            for h in range(2):
                chunks = (2 * h, 2 * h + 1)
                y_ps = psy.tile([P, W], f32, tag="y", name="y_ps")
                # software-pipelined: PE emits iteration n+1's B broadcast
                # BEFORE iteration n's y-accumulate, so the (in-order) PE
                # stream never makes the vector engine wait on the slow
                # GPSIMD h*C multiply through the Bb dependency.
                Bb, Cb = build_bb(0), build_cb(0)
                for n in range(N):
                    Bb_next = build_bb(n + 1) if n + 1 < N else None
                    Cb_next = build_cb(n + 1) if n + 1 < N else None

                    # dA over the pair, with a zeroed decay column at the
                    # pair boundary so the scan restarts for the 2nd chunk
                    dA = sp.tile([P, W], f32, tag="dA", name="dA", bufs=2)
                    nc.scalar.activation(
                        dA[:, 0:L], wide(dt_sb, chunks[0]), AF.Exp,
                        scale=A_sb[chunks[0]][:, n : n + 1],
                    )
                    nc.scalar.activation(dA[:, L : L + 1], zcol_sb[:], AF.Copy)
                    nc.scalar.activation(
                        dA[:, L + 1 : W], wide(dt_sb, chunks[1], 1, L), AF.Exp,
                        scale=A_sb[chunks[1]][:, n : n + 1],
                    )

                    dBx = sp.tile([P, W], f32, tag="dBx", name="dBx", bufs=4)
                    nc.vector.tensor_tensor(
                        dBx[:].rearrange("p (r f) -> p r f", r=2),
                        u_sb[h][:].rearrange("p (r f) -> p r f", r=2),
                        Bb[:].unsqueeze(1).broadcast_to((P, 2, L)),
                        op=OP.mult,
                    )
                    # scan in place: h overwrites dBx
                    nc.vector.tensor_tensor_scan(
                        dBx[:], dA[:], dBx[:], 0.0, op0=OP.mult, op1=OP.add
                    )
                    hC = sp.tile([P, W], F32R, tag="hC", name="hC", bufs=3)
                    hc_eng = nc.vector if n % 2 else nc.gpsimd
                    hc_eng.tensor_tensor(
                        hC[:].rearrange("p (r f) -> p r f", r=2),
                        dBx[:].rearrange("p (r f) -> p r f", r=2),
                        Cb[:].unsqueeze(1).broadcast_to((P, 2, L)),
                        op=OP.mult,
                    )
                    # y += hC via identity matmul (PSUM accumulate)
                    for tb in range(W // TB):
                        tsl = slice(tb * TB, (tb + 1) * TB)
                        _mm(nc, y_ps[:, tsl], ident_sb[:], hC[:, tsl],
                            start=(n == 0), stop=(n == N - 1),
                            f32r=USE_F32R_AUX)
                    Bb, Cb = Bb_next, Cb_next

                # yz = (y + D*xs) * silu(z)
                for dc in chunks:
                    q = (dc % 2) * L
                    nc.vector.scalar_tensor_tensor(
                        out=wide(yz_sb, dc),
                        in0=wide(xs_sb, dc),
                        scalar=D_sb[dc][:, 0:1],
                        in1=y_ps[:, q : q + L],
                        op0=OP.mult,
                        op1=OP.add,
                    )
                    nc.vector.tensor_mul(
                        wide(yz_sb, dc), wide(yz_sb, dc), wide(zs_sb, dc)
                    )

        # ---- GEMM D: out_T = W_out^T @ yz_T ----
        with tc.tile_pool(name="psD", bufs=4, space="PSUM") as psD, tc.tile_pool(
            name="osb", bufs=4
        ) as osb:
            for mb in range(DM // P):
                for tb in range(NTB):
                    ps = psD.tile([P, TB], f32, tag="psD", name="psD")
                    for dc in range(NCH):
                        _mm(
                            nc, ps[:],
                            Wout_sb[dc][:, mb * P : (mb + 1) * P],
                            wide(yz_sb, dc, tb * TB, (tb + 1) * TB),
                            start=(dc == 0), stop=(dc == NCH - 1),
                            f32r=USE_F32R_GEMM,
                        )
                    ot = osb.tile([P, TB], f32, tag="ot", name="ot")
                    nc.scalar.activation(ot[:], ps[:], AF.Copy)
                    nc.sync.dma_start(
                        io["outT"][mb * P : (mb + 1) * P, tb * TB : (tb + 1) * TB],
                        ot[:],
                    )


def build(reps=1):
    nc = bacc.Bacc(
        "TRN2",
        target_bir_lowering=False,
        debug=False,
        enable_asserts=False,
        num_devices=N_CORES,
    )
    io = {
        "xT": nc.dram_tensor("xT", (DM, L), F32, kind="ExternalInput").ap(),
        "W_in": nc.dram_tensor("W_in", (DM, 2 * DI), F32, kind="ExternalInput").ap(),
        "Wc": nc.dram_tensor("Wc", (DI, K), F32, kind="ExternalInput").ap(),
        "bconv": nc.dram_tensor("bconv", (DI, 1), F32, kind="ExternalInput").ap(),
        "Wx": nc.dram_tensor("Wx", (DI, R + 2 * N), F32, kind="ExternalInput").ap(),
        "Wdt": nc.dram_tensor("Wdt", (R, DI), F32, kind="ExternalInput").ap(),
        "bdt": nc.dram_tensor("bdt", (DI, 1), F32, kind="ExternalInput").ap(),
        "A_sc": nc.dram_tensor("A_sc", (DI, N), F32, kind="ExternalInput").ap(),
        "Dv": nc.dram_tensor("Dv", (DI, 1), F32, kind="ExternalInput").ap(),
        "W_out": nc.dram_tensor("W_out", (DI, DM), F32, kind="ExternalInput").ap(),
        "ones": nc.dram_tensor("ones", (1, P), F32R, kind="ExternalInput").ap(),
        "ident": nc.dram_tensor("ident", (P, P), F32R, kind="ExternalInput").ap(),
        "outT": nc.dram_tensor("outT", (DM, L), F32, kind="ExternalOutput").ap(),
        "dbc_dram": nc.dram_tensor("dbc_dram", (R + 2 * N, L), F32).ap(),
    }
    with tile.TileContext(nc) as tc:
        if reps == 1:
            emit_mamba(tc, io)
        else:
            with tc.For_i(0, reps, 1):
                emit_mamba(tc, io)
    nc.compile()
    return nc


_NC_CACHE = {}


def _get_nc(reps=1):
    if reps not in _NC_CACHE:
        _NC_CACHE[reps] = build(reps)
    return _NC_CACHE[reps]


def make_in_maps(inputs):
    x = np.asarray(inputs["x"], np.float32)
    in_maps = []
    for c in range(N_CORES):
        b = c % 4
        sfx = "f" if c < 4 else "b"
        xb = x[b] if c < 4 else x[b][::-1]

        def g(name):
            return np.asarray(inputs[f"{name}_{sfx}"], np.float32)

        in_maps.append(
            {
                "xT": np.ascontiguousarray(xb.T),
                "W_in": np.ascontiguousarray(g("W_in")),
                "Wc": np.ascontiguousarray(g("W_conv")),
                "bconv": np.ascontiguousarray(g("b_conv").reshape(DI, 1)),
                "Wx": np.ascontiguousarray(g("W_xproj")),
                "Wdt": np.ascontiguousarray(g("W_dt")),
                "bdt": np.ascontiguousarray(g("b_dt").reshape(DI, 1)),
                "A_sc": np.ascontiguousarray(-np.exp(g("A_log"))),
                "Dv": np.ascontiguousarray(g("D").reshape(DI, 1)),
                "W_out": np.ascontiguousarray(g("W_out")),
                "ones": np.ones((1, P), np.float32),
                "ident": np.eye(P, dtype=np.float32),
            }
        )
    return in_maps


def assemble_output(results):
    out = np.empty((4, L, DM), np.float32)
    for b in range(4):
        of = results[b]["outT"].T
        ob = results[4 + b]["outT"].T[::-1]
        out[b] = of + ob
    return out


def kernel(**inputs):
    nc = _get_nc()
    in_maps = make_in_maps(inputs)
    res = run_bass_kernel_spmd(nc, in_maps, core_ids=list(range(N_CORES)))
    return assemble_output(res.results)

